# revision 26
# baseline (speedup 1.0000x reference)
"""Trainium2 Bass kernel: per-sample position-decay mask multiply.

out[b, l, h] = data[b, l, h] * mask[b, l]
  mask[b, l] = 1 - (a_end - l)/C           if l < a_end
             = 1 - (l - a_idx)/C           elif l < sents_len
             = 0                           otherwise
  with a_end = aspect_Index + aspect_len, C = 40.

The op is pure memory-bound elementwise; the per-core DMA ceiling (16 SDMA
engines x ~26.5 GB/s) is the roofline, so the kernel minimizes bytes:

- Ragged skip: for l >= act = max(a_end, sents_len) the output is
  structurally zero; the host only ships the active prefix data[b, :act_b]
  of each sample and pre-zeroes the output buffer.
- Exact packing: each core's active positions are concatenated into one
  flat stream and laid out [128 partitions, PP positions] with no
  segment-rounding or row padding (pad < 128 positions total per core).
- fp16 wire format: inputs are rounded to fp16 on host and outputs are
  returned as fp16 (upconverted on host). Worst-case error is ~2e-3 of
  max|out|, 10x inside the 2e-2 relative-error budget, and it halves both
  HBM traffic and doubles DVE throughput.
- The position mask is precomputed on host (tiny: [128, 2*PP] fp16) and
  prepended to each partition's data region, so it rides in with tile 0's
  load — no separate small DMA. Each mask value is stored twice (pairs
  m[2l]=m[2l+1]): the DVE's 2x packed-fp16 mode requires every operand's
  innermost access-pattern dim to be stride-1 with count >= 2, which a
  plain stride-0 broadcast violates; the pair layout moves the broadcast
  to a middle dim ([(2,w),(0,50),(1,2)]) and doubles DVE throughput on
  the multiply.
- Measured ring behavior drives the DMA schedule: loads alternate across
  both HWDGE rings (a single ring of HBM reads is latency-bound), all
  stores go on the ACT ring, and tiles 0, 1, and last are small so the
  first multiplies start early and the drain tail is short.

Samples are assigned to cores by LPT (longest-processing-time greedy) on
act, so per-core totals match within a few positions; PP is shared by all
cores (SPMD: one compiled program). Compile is cached by PP.
"""

import numpy as np

import concourse.bacc as bacc
import concourse.mybir as mybir
import concourse.tile as tile
from concourse.bass_utils import run_bass_kernel_spmd

N_CORES = 8
B, L, H = 512, 512, 100
C = 40.0
NT = 8                     # tiles per core
P = 128
W_EDGE = 8                 # small first/last tiles: early store start, short tail

F16 = mybir.dt.float16


def build_bass(PP):
    """Build + compile the SPMD program for PP positions per partition."""
    nc = bacc.Bacc("TRN2", target_bir_lowering=False, debug=False)

    # per-partition layout of "data": [mask pairs (2*PP) | data (PP*H)] fp16;
    # the mask rides in with tile 0's load — no separate small DMA.
    MC = 2 * PP
    data = nc.dram_tensor("data", [P, MC + PP * H], F16, kind="ExternalInput")
    out = nc.dram_tensor("out", [P, PP * H], F16, kind="ExternalOutput")

    # Tiles 4/6 ride the SP ring; everything else, and all stores, ride
    # the ACT ring. Multiplies and stores are EMITTED in load-arrival
    # order [0,1,2,3,5,7,4,6] so the serial DVE chain never waits on a
    # late SP-ring load and the store queue is fed without gaps. Small
    # first tile (it carries the mask and starts the chain) and small
    # final tiles (short drain tail).
    if PP == 129:  # the real input distribution; hand-tuned
        widths = [8, 24, 21, 24, 21, 13, 10, 8]
    elif PP >= 4 * W_EDGE:
        mid = PP - 3 * W_EDGE
        nmid = NT - 3
        base = mid // nmid
        widths = [W_EDGE, W_EDGE] + [
            base + (1 if i < mid - base * nmid else 0) for i in range(nmid)
        ] + [W_EDGE]
    else:  # tiny-PP fallback (never hit for the real input distribution)
        n = min(NT, PP)
        widths = [PP // n + (1 if i < PP - (PP // n) * n else 0)
                  for i in range(n)]

    # Only tiles 4 and 6 (~0.8MB) ride the SP ring: its throughput is
    # erratic (95-280 GB/s between runs), and this few bytes arrives in
    # time for the multiply chain even at its observed worst.
    slow = {4, 6} if len(widths) == NT else set()
    ring = [nc.sync if i in slow else nc.scalar for i in range(len(widths))]
    exec_order = [i for i in range(len(widths)) if i not in slow] + \
                 sorted(slow)

    offs = [sum(widths[:i]) for i in range(len(widths))]

    with tile.TileContext(nc) as tc:
        with tc.tile_pool(name="io", bufs=NT) as io:
            tiles = {}
            for i in exec_order:
                w = widths[i]
                cols = (MC if i == 0 else 0) + w * H
                t = io.tile([P, cols], F16, tag="io")
                src0 = 0 if i == 0 else MC + offs[i] * H
                ring[i].dma_start(t[:], data.ap()[:, src0:src0 + cols])
                tiles[i] = t

            mask_t = tiles[0]  # mask pairs live in tile 0's first MC cols
            for i in exec_order:
                off, w = offs[i], widths[i]
                skip = MC if i == 0 else 0
                # All multiplies stay on the DVE: offloading some tiles to
                # GPSIMD was measured to wreck BOTH engines' throughput
                # (SBUF port contention on the shared mask tile).
                # view as [P, w, H/2, 2]: innermost dim is a stride-1 fp16
                # pair on all operands -> DVE 2x packed mode
                d4 = tiles[i][:, skip:skip + w * H].rearrange(
                    "p (l hh k) -> p l hh k", hh=H // 2, k=2)
                m4 = mask_t[:, 2 * off:2 * (off + w)].rearrange(
                    "p (l k) -> p l k", k=2).unsqueeze(2).broadcast_to(
                    [P, w, H // 2, 2])
                nc.vector.tensor_tensor(out=d4, in0=d4, in1=m4,
                                        op=mybir.AluOpType.mult)

            for i in exec_order:
                off, w = offs[i], widths[i]
                skip = MC if i == 0 else 0
                nc.scalar.dma_start(
                    out.ap()[:, off * H:(off + w) * H],
                    tiles[i][:, skip:skip + w * H])

    nc.compile()
    return nc


_NC_CACHE = {}


def _get_nc(PP):
    if PP not in _NC_CACHE:
        _NC_CACHE[PP] = build_bass(PP)
    return _NC_CACHE[PP]


def plan_and_pack(data, aspect_Index, aspect_len, sents_len):
    """LPT-assign samples to cores, pack each core's active prefixes into a
    flat fp16 stream [128, 2*PP + PP*H]: pair-duplicated position mask
    followed by the data."""
    data = np.asarray(data, dtype=np.float32)
    a_idx = np.asarray(aspect_Index).astype(np.int64)
    a_end = a_idx + np.asarray(aspect_len).astype(np.int64)
    s_len = np.asarray(sents_len).astype(np.int64)
    act = np.maximum(a_end, s_len)

    # LPT greedy: biggest samples first onto the least-loaded core
    order = np.argsort(-act, kind="stable")
    loads = np.zeros(N_CORES, dtype=np.int64)
    cores = [[] for _ in range(N_CORES)]
    for b in order:
        c = int(np.argmin(loads))
        loads[c] += act[b]
        cores[c].append(int(b))
    PP = max(1, -(-int(loads.max()) // P))

    # full-precision mask [B, L], computed once
    i = np.arange(L, dtype=np.float32)[None, :]
    ae = a_end[:, None].astype(np.float32)
    ai = a_idx[:, None].astype(np.float32)
    maskf = np.where(i < ae, 1.0 - (ae - i) / C,
                     np.where(i < s_len[:, None], 1.0 - (i - ai) / C,
                              0.0)).astype(np.float32)

    in_maps, recon = [], []
    for c in range(N_CORES):
        mine = cores[c]
        S = int(act[mine].sum()) if mine else 0
        buf = np.zeros((P * PP, H), dtype=np.float16)
        mk = np.zeros((P * PP, 2), dtype=np.float16)
        off = 0
        for b in mine:
            a = int(act[b])
            buf[off:off + a] = data[b, :a]      # rounds f32 -> fp16
            mk[off:off + a] = maskf[b, :a, None]  # duplicated pair layout
            off += a
        in_maps.append({"data": np.concatenate(
            [mk.reshape(P, 2 * PP), buf.reshape(P, PP * H)], axis=1)})
        recon.append((mine, S))
    return in_maps, recon, PP


def kernel(data, aspect_Index, aspect_len, sents_len):
    in_maps, recon, PP = plan_and_pack(data, aspect_Index, aspect_len,
                                       sents_len)
    a_idx = np.asarray(aspect_Index).astype(np.int64)
    a_end = a_idx + np.asarray(aspect_len).astype(np.int64)
    act = np.maximum(a_end, np.asarray(sents_len).astype(np.int64))

    nc = _get_nc(PP)
    res = run_bass_kernel_spmd(nc, in_maps, list(range(N_CORES)))

    out = np.zeros((B, L, H), dtype=np.float32)
    for c in range(N_CORES):
        mine, S = recon[c]
        r = res.results[c]["out"].reshape(P * PP, H)[:S].astype(np.float32)
        off = 0
        for b in mine:
            a = int(act[b])
            out[b, :a] = r[off:off + a]
            off += a
    return out


if __name__ == "__main__":
    rng = np.random.default_rng(1)
    d = rng.standard_normal((B, L, H), dtype=np.float32)
    ai = rng.integers(0, 100, B).astype(np.int64)
    al = rng.integers(0, 10, B).astype(np.int64)
    slv = rng.integers(0, 512, B).astype(np.int64)
    got = kernel(d, ai, al, slv)
    i = np.arange(L, dtype=np.float32)[None, :]
    ae = (ai + al).astype(np.float32)[:, None]
    aif = ai.astype(np.float32)[:, None]
    m = np.where(i < ae, 1.0 - (ae - i) / C,
                 np.where(i < slv[:, None], 1.0 - (i - aif) / C, 0.0))
    want = d * m[:, :, None].astype(np.float32)
    err = np.abs(got - want)
    print("selftest max abs err:", err.max(),
          "rel:", err.max() / np.abs(want).max())


# revision 27
# speedup vs baseline: 1.0160x; 1.0160x over previous
"""Trainium2 Bass kernel: per-sample position-decay mask multiply.

out[b, l, h] = data[b, l, h] * mask[b, l]
  mask[b, l] = 1 - (a_end - l)/C           if l < a_end
             = 1 - (l - a_idx)/C           elif l < sents_len
             = 0                           otherwise
  with a_end = aspect_Index + aspect_len, C = 40.

The op is pure memory-bound elementwise; the per-core DMA ceiling (16 SDMA
engines x ~26.5 GB/s) is the roofline, so the kernel minimizes bytes:

- Ragged skip: for l >= act = max(a_end, sents_len) the output is
  structurally zero; the host only ships the active prefix data[b, :act_b]
  of each sample and pre-zeroes the output buffer.
- Exact packing: each core's active positions are concatenated into one
  flat stream and laid out [128 partitions, PP positions] with no
  segment-rounding or row padding (pad < 128 positions total per core).
- fp16 wire format: inputs are rounded to fp16 on host and outputs are
  returned as fp16 (upconverted on host). Worst-case error is ~2e-3 of
  max|out|, 10x inside the 2e-2 relative-error budget, and it halves both
  HBM traffic and doubles DVE throughput.
- The position mask is precomputed on host (tiny: [128, 2*PP] fp16) and
  prepended to each partition's data region, so it rides in with tile 0's
  load — no separate small DMA. Each mask value is stored twice (pairs
  m[2l]=m[2l+1]): the DVE's 2x packed-fp16 mode requires every operand's
  innermost access-pattern dim to be stride-1 with count >= 2, which a
  plain stride-0 broadcast violates; the pair layout moves the broadcast
  to a middle dim ([(2,w),(0,50),(1,2)]) and doubles DVE throughput on
  the multiply.
- Measured ring behavior drives the DMA schedule: loads alternate across
  both HWDGE rings (a single ring of HBM reads is latency-bound), all
  stores go on the ACT ring, and tiles 0, 1, and last are small so the
  first multiplies start early and the drain tail is short.

Samples are assigned to cores by LPT (longest-processing-time greedy) on
act, so per-core totals match within a few positions; PP is shared by all
cores (SPMD: one compiled program). Compile is cached by PP.
"""

import numpy as np

import concourse.bacc as bacc
import concourse.mybir as mybir
import concourse.tile as tile
from concourse.bass_utils import run_bass_kernel_spmd

N_CORES = 8
B, L, H = 512, 512, 100
C = 40.0
NT = 8                     # tiles per core
KDUP = 10                  # mask duplication factor (divides H)
P = 128
W_EDGE = 8                 # small first/last tiles: early store start, short tail

F16 = mybir.dt.float16


def build_bass(PP):
    """Build + compile the SPMD program for PP positions per partition."""
    nc = bacc.Bacc("TRN2", target_bir_lowering=False, debug=False)

    # per-partition layout of "data": [mask pairs (2*PP) | data (PP*H)] fp16;
    # the mask rides in with tile 0's load — no separate small DMA.
    MC = KDUP * PP
    data = nc.dram_tensor("data", [P, MC + PP * H], F16, kind="ExternalInput")
    out = nc.dram_tensor("out", [P, PP * H], F16, kind="ExternalOutput")

    # Tiles 4/6 ride the SP ring; everything else, and all stores, ride
    # the ACT ring. Multiplies and stores are EMITTED in load-arrival
    # order [0,1,2,3,5,7,4,6] so the serial DVE chain never waits on a
    # late SP-ring load and the store queue is fed without gaps. Small
    # first tile (it carries the mask and starts the chain) and small
    # final tiles (short drain tail).
    if PP == 129:  # the real input distribution; hand-tuned
        widths = [8, 24, 21, 24, 21, 13, 10, 8]
    elif PP >= 4 * W_EDGE:
        mid = PP - 3 * W_EDGE
        nmid = NT - 3
        base = mid // nmid
        widths = [W_EDGE, W_EDGE] + [
            base + (1 if i < mid - base * nmid else 0) for i in range(nmid)
        ] + [W_EDGE]
    else:  # tiny-PP fallback (never hit for the real input distribution)
        n = min(NT, PP)
        widths = [PP // n + (1 if i < PP - (PP // n) * n else 0)
                  for i in range(n)]

    # Only tiles 4 and 6 (~0.8MB) ride the SP ring: its throughput is
    # erratic (95-280 GB/s between runs), and this few bytes arrives in
    # time for the multiply chain even at its observed worst.
    slow = {4, 6} if len(widths) == NT else set()
    ring = [nc.sync if i in slow else nc.scalar for i in range(len(widths))]
    exec_order = [i for i in range(len(widths)) if i not in slow] + \
                 sorted(slow)

    offs = [sum(widths[:i]) for i in range(len(widths))]

    with tile.TileContext(nc) as tc:
        with tc.tile_pool(name="io", bufs=NT) as io:
            tiles = {}
            for i in exec_order:
                w = widths[i]
                cols = (MC if i == 0 else 0) + w * H
                t = io.tile([P, cols], F16, tag="io")
                src0 = 0 if i == 0 else MC + offs[i] * H
                ring[i].dma_start(t[:], data.ap()[:, src0:src0 + cols])
                tiles[i] = t

            mask_t = tiles[0]  # mask pairs live in tile 0's first MC cols
            for i in exec_order:
                off, w = offs[i], widths[i]
                skip = MC if i == 0 else 0
                # All multiplies stay on the DVE: offloading some tiles to
                # GPSIMD was measured to wreck BOTH engines' throughput
                # (SBUF port contention on the shared mask tile).
                # view as [P, w, H/2, 2]: innermost dim is a stride-1 fp16
                # pair on all operands -> DVE 2x packed mode
                d4 = tiles[i][:, skip:skip + w * H].rearrange(
                    "p (l hh k) -> p l hh k", hh=H // KDUP, k=KDUP)
                m4 = mask_t[:, KDUP * off:KDUP * (off + w)].rearrange(
                    "p (l k) -> p l k", k=KDUP).unsqueeze(2).broadcast_to(
                    [P, w, H // KDUP, KDUP])
                nc.vector.tensor_tensor(out=d4, in0=d4, in1=m4,
                                        op=mybir.AluOpType.mult)

            for i in exec_order:
                off, w = offs[i], widths[i]
                skip = MC if i == 0 else 0
                nc.scalar.dma_start(
                    out.ap()[:, off * H:(off + w) * H],
                    tiles[i][:, skip:skip + w * H])

    nc.compile()
    return nc


_NC_CACHE = {}


def _get_nc(PP):
    if PP not in _NC_CACHE:
        _NC_CACHE[PP] = build_bass(PP)
    return _NC_CACHE[PP]


def plan_and_pack(data, aspect_Index, aspect_len, sents_len):
    """LPT-assign samples to cores, pack each core's active prefixes into a
    flat fp16 stream [128, 2*PP + PP*H]: pair-duplicated position mask
    followed by the data."""
    data = np.asarray(data, dtype=np.float32)
    a_idx = np.asarray(aspect_Index).astype(np.int64)
    a_end = a_idx + np.asarray(aspect_len).astype(np.int64)
    s_len = np.asarray(sents_len).astype(np.int64)
    act = np.maximum(a_end, s_len)

    # LPT greedy: biggest samples first onto the least-loaded core
    order = np.argsort(-act, kind="stable")
    loads = np.zeros(N_CORES, dtype=np.int64)
    cores = [[] for _ in range(N_CORES)]
    for b in order:
        c = int(np.argmin(loads))
        loads[c] += act[b]
        cores[c].append(int(b))
    PP = max(1, -(-int(loads.max()) // P))

    # full-precision mask [B, L], computed once
    i = np.arange(L, dtype=np.float32)[None, :]
    ae = a_end[:, None].astype(np.float32)
    ai = a_idx[:, None].astype(np.float32)
    maskf = np.where(i < ae, 1.0 - (ae - i) / C,
                     np.where(i < s_len[:, None], 1.0 - (i - ai) / C,
                              0.0)).astype(np.float32)

    in_maps, recon = [], []
    for c in range(N_CORES):
        mine = cores[c]
        S = int(act[mine].sum()) if mine else 0
        buf = np.zeros((P * PP, H), dtype=np.float16)
        mk = np.zeros((P * PP, KDUP), dtype=np.float16)
        off = 0
        for b in mine:
            a = int(act[b])
            buf[off:off + a] = data[b, :a]      # rounds f32 -> fp16
            mk[off:off + a] = maskf[b, :a, None]  # duplicated pair layout
            off += a
        in_maps.append({"data": np.concatenate(
            [mk.reshape(P, KDUP * PP), buf.reshape(P, PP * H)], axis=1)})
        recon.append((mine, S))
    return in_maps, recon, PP


def kernel(data, aspect_Index, aspect_len, sents_len):
    in_maps, recon, PP = plan_and_pack(data, aspect_Index, aspect_len,
                                       sents_len)
    a_idx = np.asarray(aspect_Index).astype(np.int64)
    a_end = a_idx + np.asarray(aspect_len).astype(np.int64)
    act = np.maximum(a_end, np.asarray(sents_len).astype(np.int64))

    nc = _get_nc(PP)
    res = run_bass_kernel_spmd(nc, in_maps, list(range(N_CORES)))

    out = np.zeros((B, L, H), dtype=np.float32)
    for c in range(N_CORES):
        mine, S = recon[c]
        r = res.results[c]["out"].reshape(P * PP, H)[:S].astype(np.float32)
        off = 0
        for b in mine:
            a = int(act[b])
            out[b, :a] = r[off:off + a]
            off += a
    return out


if __name__ == "__main__":
    rng = np.random.default_rng(1)
    d = rng.standard_normal((B, L, H), dtype=np.float32)
    ai = rng.integers(0, 100, B).astype(np.int64)
    al = rng.integers(0, 10, B).astype(np.int64)
    slv = rng.integers(0, 512, B).astype(np.int64)
    got = kernel(d, ai, al, slv)
    i = np.arange(L, dtype=np.float32)[None, :]
    ae = (ai + al).astype(np.float32)[:, None]
    aif = ai.astype(np.float32)[:, None]
    m = np.where(i < ae, 1.0 - (ae - i) / C,
                 np.where(i < slv[:, None], 1.0 - (i - aif) / C, 0.0))
    want = d * m[:, :, None].astype(np.float32)
    err = np.abs(got - want)
    print("selftest max abs err:", err.max(),
          "rel:", err.max() / np.abs(want).max())
